# revision 33
# baseline (speedup 1.0000x reference)
"""AMNet graph-matching network on 8 Trainium2 NeuronCores.

Data-parallel over the batch dim B=64: each core owns 8 graphs (1024 nodes).
Edges are random over all 8192 nodes, so:
  - node-feature tables for the per-edge gathers live in DRAM (replicated,
    locally built, or AllGathered), accessed with dma_gather;
  - each core processes exactly the edges whose *destination* node it owns
    (host pre-partitions / sorts / pads them into uniform 128-edge chunks);
  - segment-sum aggregation = per-chunk one-hot matmuls accumulated in PSUM
    (transposed: out = msg^T @ onehot -> [feat, nodes]);
  - the pairwise-difference MLP folds |W2| into W1 so the post-relu
    k-reduction becomes a +/-1-weighted PE matmul into 32-row PSUM strips.

Host side: threefry RNG for the random features (bit-identical to jax),
edge partitioning, index remapping, one-hot construction, constant folding.
"""

import numpy as np

import concourse.bass as bass
import concourse.mybir as mybir
from concourse import bacc, library_config
from concourse.tile import TileContext
from concourse.bass_utils import run_bass_kernel_spmd

F32 = mybir.dt.float32
BF16 = mybir.dt.bfloat16
I16 = mybir.dt.int16
AF = mybir.ActivationFunctionType
ALU = mybir.AluOpType
AX = mybir.AxisListType

B, N, RIN, ROUT, DEMB, FIN, FE, E, ITERS = 64, 128, 32, 64, 64, 64, 16, 65536, 3
NTOT = B * N
NCORES = 8
NG = B // NCORES          # graphs per core
LN = NG * N               # local nodes per core

BF16NP = mybir.dt.np(BF16)

_prog_cache: dict = {}
LAST_RESULTS = None
LAST_IN_MAPS = None


def _run(nc, in_maps, core_ids):
    import os
    if os.environ.get("KERNEL_SIM") == "1":
        from concourse.bass_interp import MultiCoreSim
        sim = MultiCoreSim(nc, NCORES, num_workers=int(
            os.environ.get("KERNEL_SIM_WORKERS", "8")))
        for c in range(NCORES):
            for k, v in in_maps[c].items():
                sim.cores[c].tensor(k)[:] = v
        sim.simulate()
        import concourse.mybir as mb
        out_names = [
            a.memorylocations[0].name for a in nc.m.functions[0].allocations
            if isinstance(a, mb.MemoryLocationSet) and a.kind == "ExternalOutput"
        ]
        results = [{k: np.array(sim.cores[c].tensor(k)) for k in out_names}
                   for c in range(NCORES)]

        class _R:
            pass

        r = _R()
        r.results = results
        r.exec_time_ns = None
        return r
    return run_bass_kernel_spmd(nc, in_maps, core_ids)


# ---------------------------------------------------------------- device ---

def build_program(CPB: int, apply_mask: bool, b2v: float):
    """Build the SPMD Bass program. CPB = padded 128-edge chunks per graph."""
    NCH = NG * CPB            # chunks per core per side
    EPG = CPB * 128           # padded edges per graph

    nc = bacc.Bacc(None, target_bir_lowering=False, num_devices=NCORES)

    def param(name, shape, dtype):
        return nc.declare_dram_parameter(name, list(shape), dtype, isOutput=False)

    # per-core params
    gidx_r1_d = param("gidx_r1", [128, NCH * 8], I16)
    gidx_r2_d = param("gidx_r2", [128, NCH * 8], I16)
    gidx_p1_d = param("gidx_p1", [128, NCH * 8], I16)
    gidx_p2_d = param("gidx_p2", [128, NCH * 8], I16)
    oh_r_d = param("oh_r", [128, NCH * 128], BF16)
    oh_p_d = param("oh_p", [128, NCH * 128], BF16)
    ewe1_r_d = param("ewe1_r", [128, NCH * 64], F32)
    ewe1_p_d = param("ewe1_p", [128, NCH * 64], F32)
    ewe2_r_d = param("ewe2_r", [128, NCH * 64], BF16)
    ewe2_p_d = param("ewe2_p", [128, NCH * 64], BF16)
    xTr_d = param("xTr", [FIN + 1, NTOT], F32)   # column-rolled per core
    xTp_d = param("xTp", [FIN + 1, NTOT], F32)
    rfr_d = [param(f"rfr{it}", [128, NG * RIN], F32) for it in range(ITERS)]
    rwsr_d = [param(f"rwsr{it}", [128, NG * 64], F32) for it in range(ITERS)]
    if apply_mask:
        maskp_d = param("maskp", [128, NG * 128], F32)

    # replicated params
    rwmr_d = [param(f"rwmr{it}", [NTOT, 128], BF16) for it in range(ITERS)]
    wm1_d = param("wm1", [FIN, DEMB], F32)
    ws1a_d = param("ws1a", [FIN + 1, DEMB], F32)
    wm2_d = param("wm2", [RIN, ROUT], F32)
    ws2a_d = param("ws2a", [RIN + 1, ROUT], F32)
    w1p_d = param("w1p", [ROUT, ROUT], F32)
    w1n_d = param("w1n", [ROUT, ROUT], F32)
    cp_d = param("cp", [ROUT, 1], F32)
    svec_d = param("svec", [128, 512], BF16)
    ident_d = param("ident", [128, 128], F32)

    m0_d = nc.declare_dram_parameter("m0", [128, NG * 128], F32, isOutput=True)
    mt_d = nc.declare_dram_parameter("mt", [128, NG * 128], F32, isOutput=True)

    import os
    DEBUG = os.environ.get("KERNEL_DEBUG") == "1"
    DBG_G = int(os.environ.get("KERNEL_DEBUG_G", "0"))
    if DEBUG:
        keys = [("msg", [128, CPB, 64]), ("hrT", [64, 128]),
                ("hpT", [64, 128]), ("mhat0", [128, 128]),
                ("selfx", [128, 64]), ("astk", [128, 64]),
                ("nap", [128, 128]), ("rfpT", [33, 128])]
        for it in range(ITERS):
            keys += [(f"upd{it}", [128, 128]), (f"mh{it}", [128, 128]),
                     (f"orT{it}", [64, 128]), (f"opT{it}", [64, 128]),
                     (f"msm{it}", [128, 128])]
        dbg_d = {k: nc.declare_dram_parameter(f"dbg_{k}", shp, F32,
                                              isOutput=True)
                 for k, shp in keys}

    # internal DRAM gather tables (wrapped row order)
    xwm1r_d = nc.dram_tensor("xwm1r", [NTOT, 64], F32)
    xwm1p_d = nc.dram_tensor("xwm1p", [NTOT, 64], F32)

    import os as _os
    from contextlib import ExitStack
    _lin = _os.environ.get("KERNEL_LINEARIZE") == "1"
    PHASE = _os.environ.get("KERNEL_PHASE", "full")  # front | noiter | nomlp | nogather | full
    with TileContext(nc, linearize=_lin) as tc, ExitStack() as stk:
        nc.gpsimd.load_library(library_config.mlp)

        cpool = stk.enter_context(tc.tile_pool(name="const", bufs=1))
        dpool = stk.enter_context(tc.tile_pool(name="dram", bufs=1, space="DRAM"))

        def load(pool, dram, shape, dtype, name):
            t = pool.tile(shape, dtype, name=name)
            nc.gpsimd.dma_start(out=t[:], in_=dram[:])
            return t

        gidx_r2 = load(cpool, gidx_r2_d, [128, NCH * 8], I16, "gidx_r2")
        gidx_p2 = load(cpool, gidx_p2_d, [128, NCH * 8], I16, "gidx_p2")
        oh_r = load(cpool, oh_r_d, [128, NCH * 128], BF16, "oh_r")
        oh_p = load(cpool, oh_p_d, [128, NCH * 128], BF16, "oh_p")
        ewe2_r = load(cpool, ewe2_r_d, [128, NCH * 64], BF16, "ewe2_r")
        ewe2_p = load(cpool, ewe2_p_d, [128, NCH * 64], BF16, "ewe2_p")
        wm2 = load(cpool, wm2_d, [RIN, ROUT], F32, "wm2")
        ws2a = load(cpool, ws2a_d, [RIN + 1, ROUT], F32, "ws2a")
        w1p = load(cpool, w1p_d, [ROUT, ROUT], F32, "w1p")
        w1n = load(cpool, w1n_d, [ROUT, ROUT], F32, "w1n")
        cp = load(cpool, cp_d, [ROUT, 1], F32, "cp")
        svec = load(cpool, svec_d, [128, 512], BF16, "svec")
        ident = load(cpool, ident_d, [128, 128], F32, "ident")
        rfr = [load(cpool, rfr_d[it], [128, NG * RIN], F32, f"rfr{it}")
               for it in range(ITERS)]
        rwsr = [load(cpool, rwsr_d[it], [128, NG * 64], F32, f"rwsr{it}")
                for it in range(ITERS)]
        if apply_mask:
            maskp = load(cpool, maskp_d, [128, NG * 128], F32, "maskp")

        # M_hat tiles (SBUF-resident, f32)
        mhat = [cpool.tile([128, 128], F32, name=f"mhat{g}") for g in range(NG)]

        # psum pools (each slot padded to one 2KB bank; 8 banks total)
        psS = stk.enter_context(tc.tile_pool(name="psS", bufs=2, space="PSUM"))
        psAgg = stk.enter_context(tc.tile_pool(name="psAgg", bufs=2, space="PSUM"))
        psU = stk.enter_context(tc.tile_pool(name="psU", bufs=2, space="PSUM"))
        psD = stk.enter_context(tc.tile_pool(name="psD", bufs=2, space="PSUM"))

        # work pools
        gath_p = stk.enter_context(tc.tile_pool(name="gath", bufs=3))
        msg_p = stk.enter_context(tc.tile_pool(name="msg", bufs=3))
        oT_p = stk.enter_context(tc.tile_pool(name="oT", bufs=9))
        small_p = stk.enter_context(tc.tile_pool(name="small", bufs=9))
        big_p = stk.enter_context(tc.tile_pool(name="big", bufs=2))
        rt_p = stk.enter_context(tc.tile_pool(name="rt", bufs=4))

        def gnn_pass(table_d, gidx_t, ewe_t, oh_t, self_aps, name,
                     f32=False, oh_cast_pool=None, ewe_from_dram=False,
                     dump_key=None):
            """One GNN layer over this core's 8 graphs.

            self_aps[g]: SBUF AP [128, 64] f32 = (x @ Ws + b) rows of graph g.
            f32: full-precision path (gnn1, feeds the huge M_hat logits);
                 gathers a [NTOT, 64] f32 table and casts one-hots to f32.
            Returns list of f32 [64, 128] transposed node outputs.
            """
            outs = []
            dt = F32 if f32 else BF16
            ew = 64 if f32 else 128
            if not ewe_from_dram:
                ewe_ap = ewe_t[:].rearrange("p (c f) -> p c f", f=64)
            NOGATHER = _os.environ.get("KERNEL_NOGATHER") == "1"
            NOMM = _os.environ.get("KERNEL_NOMM") == "1"
            for g in range(NG):
                gath = gath_p.tile([128, CPB, ew], dt,
                                   tag="gathf" if f32 else "gath")
                if NOGATHER:
                    nc.vector.memset(gath[:], 0.0)
                else:
                    nc.gpsimd.dma_gather(
                        gath[:], table_d[:],
                        gidx_t[:, g * CPB * 8:(g + 1) * CPB * 8],
                        num_idxs=EPG, num_idxs_reg=EPG, elem_size=ew,
                        single_packet=False,
                    )
                msg = msg_p.tile([128, CPB, 64], dt,
                                 tag="msgf" if f32 else "msg")
                if ewe_from_dram:
                    ewt = oh_cast_pool.tile([128, CPB, 64], F32, tag="ewt",
                                            bufs=2)
                    nc.gpsimd.dma_start(
                        out=ewt[:],
                        in_=ewe_t[:, g * CPB * 64:(g + 1) * CPB * 64]
                        .rearrange("p (c f) -> p c f", f=64))
                    ewe_g = ewt[:]
                else:
                    ewe_g = ewe_ap[:, g * CPB:(g + 1) * CPB, :]
                nc.vector.tensor_tensor(
                    msg[:], gath[:, :, 0:64], ewe_g, op=ALU.add,
                )
                nc.vector.tensor_scalar(msg[:], msg[:], 0.0, None, op0=ALU.max)
                if DEBUG and name == "hr" and g == DBG_G:
                    nc.gpsimd.dma_start(out=dbg_d["msg"][:], in_=msg[:])
                if f32:
                    ohf = oh_cast_pool.tile([128, CPB * 128], F32, tag="ohf",
                                            bufs=2)
                    nc.vector.tensor_copy(
                        ohf[:], oh_t[:, g * CPB * 128:(g + 1) * CPB * 128])
                    oh_src = ohf
                    oh_off = 0
                else:
                    oh_src = oh_t
                    oh_off = g * CPB * 128
                agg = psAgg.tile([64, 128], F32, tag="agg")
                nc.tensor.matmul(agg[:], self_aps[g], ident[:],
                                 start=True, stop=False)
                for c in range(CPB if not NOMM else 1):
                    nc.tensor.matmul(
                        agg[:], msg[:, c, :],
                        oh_src[:, oh_off + c * 128:oh_off + (c + 1) * 128],
                        start=False, stop=(c == (CPB - 1 if not NOMM else 0)),
                    )
                oT = oT_p.tile([64, 128], F32, tag=f"oT{name}")
                nc.scalar.activation(oT[:], agg[:], AF.Relu)
                outs.append(oT)
                if DEBUG and g == DBG_G and dump_key is not None:
                    nc.gpsimd.dma_start(out=dbg_d[dump_key][:], in_=oT[:])
            return outs

        # ------------------------------------------------ startup + gnn1 ---
        with tc.tile_pool(name="tmp1", bufs=1) as tpool:
            gidx_r1 = load(tpool, gidx_r1_d, [128, NCH * 8], I16, "gidx_r1")
            gidx_p1 = load(tpool, gidx_p1_d, [128, NCH * 8], I16, "gidx_p1")
            wm1 = load(tpool, wm1_d, [FIN, DEMB], F32, "wm1")
            ws1a = load(tpool, ws1a_d, [FIN + 1, DEMB], F32, "ws1a")

            # XWm1 gather tables in DRAM, wrapped row order row'=(p*64+a).
            # xT is column-rolled per core, so the table (and gidx_*1) are in
            # rolled node order; gather results are order-insensitive.
            # The two sides share one xT slot (tag) to halve SBUF.
            selfx = {}
            for side, xT_d, tab_d in (("r", xTr_d, xwm1r_d),
                                      ("p", xTp_d, xwm1p_d)):
                xTs = tpool.tile([FIN + 1, NTOT], F32, tag="xT",
                                 name=f"xT{side}")
                nc.gpsimd.dma_start(out=xTs[:], in_=xT_d[:])
                tab_ap = tab_d[:].rearrange("(p a) f -> p a f", p=128)
                for grp in range(8):
                    ps = psS.tile([128, 512], F32, tag="psS")
                    for j in range(8):
                        a = grp * 8 + j
                        nc.tensor.matmul(
                            ps[:, 64 * j:64 * j + 64],
                            xTs[0:FIN, 128 * a:128 * a + 128], wm1[:],
                            start=True, stop=True,
                        )
                    tg = tpool.tile([128, 8, 64], F32, tag="tabg", bufs=2)
                    nc.vector.tensor_copy(
                        tg[:], ps[:].rearrange("p (a f) -> p a f", f=64))
                    nc.gpsimd.dma_start(
                        out=tab_ap[:, grp * 8:grp * 8 + 8, :], in_=tg[:])

                # XWs1 = x @ Ws1 + b for the core's own blocks (front columns)
                lst = []
                for g in range(NG):
                    ps = psS.tile([128, 512], F32, tag="psS")
                    nc.tensor.matmul(
                        ps[:, 0:64], xTs[:, g * 128:(g + 1) * 128],
                        ws1a[:], start=True, stop=True,
                    )
                    sx = tpool.tile([128, 64], F32, name=f"sx{side}{g}")
                    nc.scalar.copy(sx[:], ps[:, 0:64])
                    if DEBUG and side == "r" and g == DBG_G:
                        nc.gpsimd.dma_start(out=dbg_d["selfx"][:], in_=sx[:])
                    lst.append(sx)
                selfx[side] = lst

            if PHASE == "tables":
                for g in range(NG):
                    nc.vector.memset(mhat[g][:], 0.0)
            else:
                hr = gnn_pass(xwm1r_d, gidx_r1, ewe1_r_d, oh_r,
                              [t[:] for t in selfx["r"]], "hr",
                              f32=True, oh_cast_pool=tpool, ewe_from_dram=True,
                              dump_key="hrT")
                if PHASE == "gnn1r":
                    for g in range(NG):
                        ps = psS.tile([128, 512], F32, tag="psS")
                        nc.tensor.matmul(ps[:, 0:128], hr[g][:], hr[g][:],
                                         start=True, stop=True)
                        nc.vector.tensor_copy(mhat[g][:], ps[:, 0:128])
                else:
                    hp = gnn_pass(xwm1p_d, gidx_p1, ewe1_p_d, oh_p,
                                  [t[:] for t in selfx["p"]], "hp",
                                  f32=True, oh_cast_pool=tpool,
                                  ewe_from_dram=True, dump_key="hpT")
                    for g in range(NG):
                        ps = psS.tile([128, 512], F32, tag="psS")
                        nc.tensor.matmul(ps[:, 0:128], hr[g][:], hp[g][:],
                                         start=True, stop=True)
                        nc.vector.tensor_copy(mhat[g][:], ps[:, 0:128])
                if DEBUG and g == DBG_G:
                    nc.gpsimd.dma_start(out=dbg_d["mhat0"][:], in_=mhat[g][:])

        # ---------------------------------------------------- iterations ---
        def softmax(g, tag):
            negmx = small_p.tile([128, 1], F32, tag="negmx", bufs=40)
            nc.vector.tensor_reduce(negmx[:], mhat[g][:], axis=AX.X,
                                    op=ALU.max, negate=True)
            msm = big_p.tile([128, 128], F32, tag="msm", bufs=9)
            se = small_p.tile([128, 1], F32, tag="se", bufs=40)
            nc.scalar.activation(msm[:], mhat[g][:], AF.Exp,
                                 bias=negmx[:], scale=1.0, accum_out=se[:])
            rs = small_p.tile([128, 1], F32, tag="rs", bufs=40)
            nc.vector.reciprocal(rs[:], se[:])
            nc.vector.tensor_scalar(msm[:], msm[:], rs[:], None, op0=ALU.mult)
            return msm

        for it in range(ITERS if PHASE in ("full", "nomlp") else 0):
            agin_sb = big_p.tile([128, NG, 128], BF16, tag="agin", bufs=2)
            nc.vector.memset(agin_sb[:, :, 64:128], 0.0)
            selfp = []
            for g in range(NG):
                msm = softmax(g, "msm")
                if it == 0:
                    nc.gpsimd.dma_start(out=m0_d[:, g * 128:(g + 1) * 128],
                                      in_=msm[:])
                # rf_pT = rf_r^T @ M  -> [32, 128]
                psT = psS.tile([128, 512], F32, tag="psS")
                nc.tensor.matmul(
                    psT[0:RIN, 0:128],
                    rfr[it][:].rearrange("p (g i) -> p g i", i=RIN)[:, g, :],
                    msm[:], start=True, stop=True,
                )
                rfpT = small_p.tile([RIN + 1, 128], F32, tag="rfpT")
                nc.vector.memset(rfpT[RIN:RIN + 1, :], 1.0)
                nc.scalar.copy(rfpT[0:RIN, :], psT[0:RIN, 0:128])
                if DEBUG and it == 0 and g == DBG_G:
                    nc.gpsimd.dma_start(out=dbg_d["rfpT"][:], in_=rfpT[:])
                if DEBUG and g == DBG_G:
                    nc.gpsimd.dma_start(out=dbg_d[f"msm{it}"][:], in_=msm[:])
                # RWm_p local -> AllGather input (bf16, 128-col padded)
                ps1 = psS.tile([128, 512], F32, tag="psS")
                nc.tensor.matmul(ps1[:, 0:64], rfpT[0:RIN, :], wm2[:],
                                 start=True, stop=True)
                nc.scalar.copy(agin_sb[:, g, 0:64], ps1[:, 0:64])
                # RWs_p local (self term for gnn2-p)
                ps2 = psS.tile([128, 512], F32, tag="psS")
                nc.tensor.matmul(ps2[:, 0:64], rfpT[:], ws2a[:],
                                 start=True, stop=True)
                sp = small_p.tile([128, 64], F32, tag="selfp")
                nc.scalar.copy(sp[:], ps2[:, 0:64])
                selfp.append(sp)

            agin_dram = dpool.tile([128, NG * 128], BF16, name=f"agin{it}")
            nc.gpsimd.dma_start(out=agin_dram[:],
                              in_=agin_sb[:].rearrange("p g f -> p (g f)"))
            rwmp_dram = dpool.tile([NTOT, 128], BF16, name=f"rwmp{it}")
            nc.gpsimd.collective_compute(
                "AllGather", ALU.bypass,
                replica_groups=[list(range(NCORES))],
                ins=[agin_dram[:]], outs=[rwmp_dram[:]],
            )

            # r-side gnn2 (independent of the AllGather; overlaps it)
            rws_ap = rwsr[it][:].rearrange("p (g f) -> p g f", f=64)
            orT = gnn_pass(rwmr_d[it], gidx_r2, ewe2_r, oh_r,
                           [rws_ap[:, g, :] for g in range(NG)], "or",
                           dump_key=f"orT{it}")
            astk = []
            for g in range(NG):
                psA = psS.tile([128, 512], F32, tag="psS")
                nc.tensor.matmul(psA[0:64, 0:128], w1p[:], orT[g][:],
                                 start=True, stop=True)
                st = small_p.tile([128, 64], F32, tag="astk", bufs=32)
                pairs = psA[0:64, 0:128].rearrange("k (u two) -> k u two", two=2)
                nc.scalar.activation(st[0:64, :], pairs[:, :, 0],
                                     AF.Identity, bias=cp[:])
                nc.scalar.activation(st[64:128, :], pairs[:, :, 1],
                                     AF.Identity, bias=cp[:])
                if DEBUG and it == 0 and g == DBG_G:
                    nc.gpsimd.dma_start(out=dbg_d["astk"][:], in_=st[:])
                astk.append(st)

            # p-side gnn2 (waits for the AllGather)
            opT = gnn_pass(rwmp_dram, gidx_p2, ewe2_p, oh_p,
                           [t[:] for t in selfp], "op",
                           dump_key=f"opT{it}")

            for g in range(NG):
                psd = psD.tile([128, 128], F32, tag="psD")
                nc.tensor.matmul(psd[0:64, :], w1n[:], opT[g][:],
                                 start=True, stop=True, tile_position=(0, 0))
                nc.tensor.matmul(psd[64:128, :], w1n[:], opT[g][:],
                                 start=True, stop=True, tile_position=(0, 64))
                nap = big_p.tile([128, 128], BF16, tag="nap", bufs=2)
                nc.scalar.copy(nap[:], psd[:])
                if DEBUG and it == 0 and g == DBG_G:
                    nc.gpsimd.dma_start(out=dbg_d["nap"][:], in_=nap[:])

                psu = psU.tile([128, 128], F32, tag="psU")
                for u in range(0 if PHASE == "nomlp" else 64):
                    rt = rt_p.tile([128, 128], BF16, tag="rt")
                    nc.vector.tensor_scalar(
                        rt[:], nap[:], astk[g][:, u:u + 1], 0.0,
                        op0=ALU.add, op1=ALU.max,
                    )
                    strip, t = divmod(u, 16)
                    nc.tensor.matmul(
                        psu[32 * strip:32 * strip + 32, :],
                        svec[:, 32 * t:32 * t + 32], rt[:],
                        start=(t == 0), stop=(t == 15),
                        tile_position=(0, 32 * strip),
                    )
                if DEBUG and g == DBG_G:
                    du = big_p.tile([128, 128], F32, tag="dbgupd", bufs=1)
                    nc.vector.tensor_copy(du[:], psu[:])
                    nc.gpsimd.dma_start(out=dbg_d[f"upd{it}"][:], in_=du[:])
                # M_hat += upd  (optionally masked / +b2)
                if PHASE == "nomlp":
                    continue
                if b2v != 0.0 or apply_mask:
                    upd = big_p.tile([128, 128], F32, tag="upd", bufs=2)
                    if b2v != 0.0:
                        nc.vector.tensor_scalar(upd[:], psu[:], b2v, None,
                                                op0=ALU.add)
                    else:
                        nc.vector.tensor_copy(upd[:], psu[:])
                    if apply_mask:
                        nc.vector.tensor_tensor(
                            upd[:], upd[:],
                            maskp[:, g * 128:(g + 1) * 128], op=ALU.mult)
                    nc.vector.tensor_tensor(mhat[g][:], mhat[g][:], upd[:],
                                            op=ALU.add)
                else:
                    nc.vector.tensor_tensor(mhat[g][:], mhat[g][:], psu[:],
                                            op=ALU.add)
                if DEBUG and g == DBG_G:
                    nc.gpsimd.dma_start(out=dbg_d[f"mh{it}"][:],
                                        in_=mhat[g][:])

        for g in range(NG):
            msm = softmax(g, "msmf")
            nc.gpsimd.dma_start(out=mt_d[:, g * 128:(g + 1) * 128], in_=msm[:])

    nc.compile()
    return nc


# ------------------------------------------------------------------ host ---

def _wrap_idx(idx_flat: np.ndarray) -> np.ndarray:
    """[n] -> [128, n//16] wrapped (i -> [i%16, i//16]), 8x replicated."""
    w = idx_flat.reshape(-1, 16).T.astype(np.int16)
    return np.tile(w, (8, 1))


def _count_cpb(edge_index) -> int:
    dst = np.asarray(edge_index[1], dtype=np.int64) % NTOT
    counts = np.bincount(dst // 128, minlength=B)
    return max(1, int(np.ceil(counts.max() / 128)))


def _prep_side(edge_index, edge_feat, We1, We2, CPB):
    """Partition/sort/pad edges by dst owner. Returns list of per-core dicts."""
    src = np.asarray(edge_index[0], dtype=np.int64) % NTOT
    dst = np.asarray(edge_index[1], dtype=np.int64) % NTOT
    ef = np.asarray(edge_feat, dtype=np.float32)
    EPG = CPB * 128
    NCH = NG * CPB
    out = []
    for c in range(NCORES):
        sel = (dst // LN) == c
        s_c, d_c, f_c = src[sel], dst[sel] - c * LN, ef[sel]
        order = np.argsort(d_c, kind="stable")
        s_c, d_c, f_c = s_c[order], d_c[order], f_c[order]
        srcs = np.zeros(NG * EPG, np.int64)
        dloc = np.zeros(NG * EPG, np.int64)
        valid = np.zeros(NG * EPG, bool)
        efp = np.zeros((NG * EPG, FE), np.float32)
        for g in range(NG):
            mk = (d_c // 128) == g
            k = int(mk.sum())
            assert k <= EPG
            srcs[g * EPG:g * EPG + k] = s_c[mk]
            dloc[g * EPG:g * EPG + k] = d_c[mk] % 128
            valid[g * EPG:g * EPG + k] = True
            efp[g * EPG:g * EPG + k] = f_c[mk]
        oh = np.zeros((NCH, 128, 128), np.float32)
        ch = np.arange(NG * EPG) // 128
        pos = np.arange(NG * EPG) % 128
        oh[ch[valid], pos[valid], dloc[valid]] = 1.0
        oh_flat = oh.transpose(1, 0, 2).reshape(128, NCH * 128)
        ewe1 = (efp @ We1).reshape(NCH, 128, 64).transpose(1, 0, 2)
        ewe2 = (efp @ We2).reshape(NCH, 128, 64).transpose(1, 0, 2)
        _ = None
        # index remaps into the three table layouts
        rolled = (srcs - c * LN) % NTOT
        roll_wrap = (rolled % 128) * (NTOT // 128) + rolled // 128
        full_wrap = (srcs % 128) * (NTOT // 128) + srcs // 128
        l = srcs % LN
        ag_wrap = (srcs // LN) * LN + (l % 128) * NG + l // 128

        def wrap_all(v):
            return np.hstack([_wrap_idx(v[g * EPG:(g + 1) * EPG])
                              for g in range(NG)])

        out.append(dict(
            gidx_roll=wrap_all(roll_wrap),
            gidx_full=wrap_all(full_wrap),
            gidx_ag=wrap_all(ag_wrap),
            oh=oh_flat.astype(BF16NP),
            ewe1=np.ascontiguousarray(ewe1.reshape(128, NCH * 64),
                                      dtype=np.float32),
            ewe2=ewe2.reshape(128, NCH * 64).astype(BF16NP),
        ))
    return out


_RF_CACHE = None

_RF_CODE = """
import os
os.environ["JAX_PLATFORMS"] = "cpu"
import numpy as np
import jax
rkey = jax.random.key(42)
out = {}
for it in range(%d):
    out[f"rf{it}"] = np.asarray(jax.random.normal(
        jax.random.fold_in(rkey, it), (%d, %d, %d), jax.numpy.float32))
np.savez(%r, **out)
"""


def _gen_rf():
    """Random features, bit-identical to the reference's jax-on-CPU threefry.

    Must run in a subprocess with JAX_PLATFORMS=cpu: with the axon/neuron
    jax plugin loaded, jax.random produces different bits even under
    jax.default_device(cpu).
    """
    global _RF_CACHE
    if _RF_CACHE is not None:
        return _RF_CACHE
    import os
    import subprocess
    import sys
    import tempfile
    import time
    path = os.path.join(tempfile.mkdtemp(), "rf.npz")
    code = _RF_CODE % (ITERS, B, N, RIN, path)
    env = dict(os.environ)
    env["JAX_PLATFORMS"] = "cpu"
    last = None
    for attempt in range(4):
        r = subprocess.run([sys.executable, "-c", code], env=env,
                           capture_output=True, text=True)
        if r.returncode == 0 and os.path.exists(path):
            break
        last = r.stderr[-1000:]
        time.sleep(2.0 * (attempt + 1))
    else:
        raise RuntimeError(f"rf generation subprocess failed: {last}")
    d = np.load(path)
    _RF_CACHE = [d[f"rf{it}"] for it in range(ITERS)]
    return _RF_CACHE


def _full_wrap_rows(T: np.ndarray) -> np.ndarray:
    """[NTOT, f] -> wrapped row order row' = (r%128)*(NTOT//128) + r//128."""
    return T.reshape(NTOT // 128, 128, -1).transpose(1, 0, 2).reshape(NTOT, -1)


def _pad128(T: np.ndarray) -> np.ndarray:
    out = np.zeros((T.shape[0], 128), np.float32)
    out[:, :T.shape[1]] = T
    return out


def kernel(x_r, edge_index_r, edge_feat_r, x_p, edge_index_p, edge_feat_p,
           r_mask, p_mask,
           g1_Ws, g1_Wm, g1_We, g1_b,
           g2_Ws, g2_Wm, g2_We, g2_b,
           mlp_W1, mlp_b1, mlp_W2, mlp_b2):
    x_r = np.asarray(x_r, np.float32)
    x_p = np.asarray(x_p, np.float32)
    g1_Ws, g1_Wm, g1_We, g1_b = [np.asarray(a, np.float32) for a in
                                 (g1_Ws, g1_Wm, g1_We, g1_b)]
    g2_Ws, g2_Wm, g2_We, g2_b = [np.asarray(a, np.float32) for a in
                                 (g2_Ws, g2_Wm, g2_We, g2_b)]
    mlp_W1, mlp_b1, mlp_W2, mlp_b2 = [np.asarray(a, np.float32) for a in
                                      (mlp_W1, mlp_b1, mlp_W2, mlp_b2)]
    r_mask = np.asarray(r_mask, bool)
    p_mask = np.asarray(p_mask, bool)

    rfs = _gen_rf()

    CPB = max(_count_cpb(edge_index_r), _count_cpb(edge_index_p))
    side_r = _prep_side(edge_index_r, edge_feat_r, g1_We, g2_We, CPB)
    side_p = _prep_side(edge_index_p, edge_feat_p, g1_We, g2_We, CPB)

    apply_mask = not (r_mask.all() and p_mask.all())
    b2v = float(mlp_b2[0])

    key = (CPB, apply_mask, b2v != 0.0)
    if key not in _prog_cache:
        _prog_cache[key] = build_program(CPB, apply_mask, b2v)
    nc = _prog_cache[key]

    # --- folded MLP constants ---
    aw = np.abs(mlp_W2[:, 0])
    s = np.sign(mlp_W2[:, 0]).astype(np.float32)
    s[s == 0] = 1.0
    w1p = mlp_W1 * aw[None, :]
    cp = (mlp_b1 * aw)[:, None].astype(np.float32)
    svec = np.zeros((128, 512), np.float32)
    for t in range(16):
        svec[0:64, 32 * t + 2 * t] = s
        svec[64:128, 32 * t + 2 * t + 1] = s

    rwmr, rws_flat, rfr_flat = [], [], []
    for it in range(ITERS):
        rf_flat = rfs[it].reshape(NTOT, RIN)
        rwmr.append(_pad128(_full_wrap_rows(rf_flat @ g2_Wm)).astype(BF16NP))
        rws_flat.append(rf_flat @ g2_Ws + g2_b[None, :])
        rfr_flat.append(rf_flat)

    rep = {
        "wm1": g1_Wm, "ws1a": np.vstack([g1_Ws, g1_b[None, :]]),
        "wm2": g2_Wm, "ws2a": np.vstack([g2_Ws, g2_b[None, :]]),
        "w1p": w1p.astype(np.float32), "w1n": (-w1p).astype(np.float32),
        "cp": cp, "svec": svec.astype(BF16NP),
        "ident": np.eye(128, dtype=np.float32),
    }
    for it in range(ITERS):
        rep[f"rwmr{it}"] = rwmr[it]

    in_maps = []
    for c in range(NCORES):
        m = dict(rep)
        for side, x in (("r", x_r), ("p", x_p)):
            xro = np.roll(x, -c * LN, axis=0)
            m[f"xT{side}"] = np.ascontiguousarray(
                np.vstack([xro.T, np.ones((1, NTOT), np.float32)]))
        d_r, d_p = side_r[c], side_p[c]
        m["gidx_r1"] = d_r["gidx_roll"]
        m["gidx_r2"] = d_r["gidx_full"]
        m["gidx_p1"] = d_p["gidx_roll"]
        m["gidx_p2"] = d_p["gidx_ag"]
        m["oh_r"] = d_r["oh"]
        m["oh_p"] = d_p["oh"]
        m["ewe1_r"] = d_r["ewe1"]
        m["ewe1_p"] = d_p["ewe1"]
        m["ewe2_r"] = d_r["ewe2"]
        m["ewe2_p"] = d_p["ewe2"]
        loc = slice(c * LN, (c + 1) * LN)
        for it in range(ITERS):
            m[f"rfr{it}"] = np.ascontiguousarray(
                rfr_flat[it][loc].reshape(NG, 128, RIN)
                .transpose(1, 0, 2).reshape(128, NG * RIN)).astype(np.float32)
            m[f"rwsr{it}"] = np.ascontiguousarray(
                rws_flat[it][loc].reshape(NG, 128, 64)
                .transpose(1, 0, 2).reshape(128, NG * 64)).astype(np.float32)
        if apply_mask:
            rm = r_mask[c * NG:(c + 1) * NG].astype(np.float32)  # [NG, 128]
            pm = p_mask[c * NG:(c + 1) * NG].astype(np.float32)
            mp = rm[:, :, None] * pm[:, None, :]                 # [NG, r, p]
            m["maskp"] = np.ascontiguousarray(
                mp.transpose(1, 0, 2).reshape(128, NG * 128))
        in_maps.append(m)

    global LAST_RESULTS, LAST_IN_MAPS
    LAST_IN_MAPS = in_maps
    res = _run(nc, in_maps, list(range(NCORES)))
    LAST_RESULTS = res
    m0 = np.concatenate(
        [np.asarray(res.results[c]["m0"]).reshape(128, NG, 128)
         .transpose(1, 0, 2) for c in range(NCORES)], axis=0).reshape(NTOT, N)
    mt = np.concatenate(
        [np.asarray(res.results[c]["mt"]).reshape(128, NG, 128)
         .transpose(1, 0, 2) for c in range(NCORES)], axis=0).reshape(NTOT, N)
    return (np.ascontiguousarray(m0, dtype=np.float32),
            np.ascontiguousarray(mt, dtype=np.float32))


# revision 42
# speedup vs baseline: 16.6888x; 16.6888x over previous
"""AMNet graph-matching network on 8 Trainium2 NeuronCores.

Data-parallel over the batch dim B=64: each core owns 8 graphs (1024 nodes).
Edges are random over all 8192 nodes, so:
  - node-feature tables for the per-edge gathers live in DRAM (replicated,
    locally built, or AllGathered), accessed with dma_gather;
  - each core processes exactly the edges whose *destination* node it owns
    (host pre-partitions / sorts / pads them into uniform 128-edge chunks);
  - segment-sum aggregation = per-chunk one-hot matmuls accumulated in PSUM
    (transposed: out = msg^T @ onehot -> [feat, nodes]);
  - the pairwise-difference MLP folds |W2| into W1 so the post-relu
    k-reduction becomes a +/-1-weighted PE matmul into 32-row PSUM strips.

Host side: threefry RNG for the random features (bit-identical to jax),
edge partitioning, index remapping, one-hot construction, constant folding.
"""

import numpy as np

import concourse.bass as bass
import concourse.mybir as mybir
from concourse import bacc, library_config
from concourse.tile import TileContext
from concourse.bass_utils import run_bass_kernel_spmd

F32 = mybir.dt.float32
BF16 = mybir.dt.bfloat16
I16 = mybir.dt.int16
AF = mybir.ActivationFunctionType
ALU = mybir.AluOpType
AX = mybir.AxisListType

B, N, RIN, ROUT, DEMB, FIN, FE, E, ITERS = 64, 128, 32, 64, 64, 64, 16, 65536, 3
NTOT = B * N
NCORES = 8
NG = B // NCORES          # graphs per core
LN = NG * N               # local nodes per core

BF16NP = mybir.dt.np(BF16)

_prog_cache: dict = {}
LAST_RESULTS = None
LAST_IN_MAPS = None


def _run(nc, in_maps, core_ids):
    import os
    if os.environ.get("KERNEL_SIM") == "1":
        from concourse.bass_interp import MultiCoreSim
        sim = MultiCoreSim(nc, NCORES, num_workers=int(
            os.environ.get("KERNEL_SIM_WORKERS", "8")))
        for c in range(NCORES):
            for k, v in in_maps[c].items():
                sim.cores[c].tensor(k)[:] = v
        sim.simulate()
        import concourse.mybir as mb
        out_names = [
            a.memorylocations[0].name for a in nc.m.functions[0].allocations
            if isinstance(a, mb.MemoryLocationSet) and a.kind == "ExternalOutput"
        ]
        results = [{k: np.array(sim.cores[c].tensor(k)) for k in out_names}
                   for c in range(NCORES)]

        class _R:
            pass

        r = _R()
        r.results = results
        r.exec_time_ns = None
        return r
    return run_bass_kernel_spmd(nc, in_maps, core_ids)


# ---------------------------------------------------------------- device ---

def build_program(CPB: int, apply_mask: bool, b2v: float):
    """Build the SPMD Bass program. CPB = padded 128-edge chunks per graph."""
    NCH = NG * CPB            # chunks per core per side
    EPG = CPB * 128           # padded edges per graph

    nc = bacc.Bacc(None, target_bir_lowering=False, num_devices=NCORES)

    def param(name, shape, dtype):
        return nc.declare_dram_parameter(name, list(shape), dtype, isOutput=False)

    # per-core params
    gidx_r1_d = param("gidx_r1", [128, NCH * 8], I16)
    gidx_r2_d = param("gidx_r2", [128, NCH * 8], I16)
    gidx_p1_d = param("gidx_p1", [128, NCH * 8], I16)
    gidx_p2_d = param("gidx_p2", [128, NCH * 8], I16)
    oh_r_d = param("oh_r", [128, NCH * 128], BF16)
    oh_p_d = param("oh_p", [128, NCH * 128], BF16)
    ewe1_r_d = param("ewe1_r", [128, NCH * 64], F32)
    ewe1_p_d = param("ewe1_p", [128, NCH * 64], F32)
    ewe2_r_d = param("ewe2_r", [128, NCH * 64], BF16)
    ewe2_p_d = param("ewe2_p", [128, NCH * 64], BF16)
    xTr_d = param("xTr", [FIN + 1, NTOT], F32)   # column-rolled per core
    xTp_d = param("xTp", [FIN + 1, NTOT], F32)
    rfr_d = [param(f"rfr{it}", [128, NG * RIN], F32) for it in range(ITERS)]
    rwsr_d = [param(f"rwsr{it}", [128, NG * 64], F32) for it in range(ITERS)]
    if apply_mask:
        maskp_d = param("maskp", [128, NG * 128], F32)

    # replicated params
    rwmr_d = [param(f"rwmr{it}", [NTOT, 128], BF16) for it in range(ITERS)]
    wm1_d = param("wm1", [FIN, DEMB], F32)
    ws1a_d = param("ws1a", [FIN + 1, DEMB], F32)
    wm2_d = param("wm2", [RIN, ROUT], F32)
    ws2a_d = param("ws2a", [RIN + 1, ROUT], F32)
    w1p_d = param("w1p", [ROUT, ROUT], F32)
    w1n_d = param("w1n", [ROUT, ROUT], F32)
    cp_d = param("cp", [ROUT, 1], F32)
    svec_d = param("svec", [128, 512], BF16)
    ident_d = param("ident", [128, 128], F32)

    m0_d = nc.declare_dram_parameter("m0", [128, NG * 128], F32, isOutput=True)
    mt_d = nc.declare_dram_parameter("mt", [128, NG * 128], F32, isOutput=True)

    import os
    DEBUG = os.environ.get("KERNEL_DEBUG") == "1"
    DBG_G = int(os.environ.get("KERNEL_DEBUG_G", "0"))
    if DEBUG:
        keys = [("msg", [128, CPB, 64]), ("hrT", [64, 128]),
                ("hpT", [64, 128]), ("mhat0", [128, 128]),
                ("selfx", [128, 64]), ("astk", [128, 64]),
                ("nap", [128, 128]), ("rfpT", [33, 128])]
        for it in range(ITERS):
            keys += [(f"upd{it}", [128, 128]), (f"mh{it}", [128, 128]),
                     (f"orT{it}", [64, 128]), (f"opT{it}", [64, 128]),
                     (f"msm{it}", [128, 128])]
        dbg_d = {k: nc.declare_dram_parameter(f"dbg_{k}", shp, F32,
                                              isOutput=True)
                 for k, shp in keys}

    # internal DRAM gather tables (wrapped row order)
    xwm1r_d = nc.dram_tensor("xwm1r", [NTOT, 64], F32)
    xwm1p_d = nc.dram_tensor("xwm1p", [NTOT, 64], F32)

    import os as _os
    from contextlib import ExitStack
    _lin = _os.environ.get("KERNEL_LINEARIZE") == "1"
    PHASE = _os.environ.get("KERNEL_PHASE", "full")  # front | noiter | nomlp | nogather | full
    with TileContext(nc, linearize=_lin) as tc, ExitStack() as stk:
        nc.gpsimd.load_library(library_config.mlp)

        cpool = stk.enter_context(tc.tile_pool(name="const", bufs=1))
        dpool = stk.enter_context(tc.tile_pool(name="dram", bufs=1, space="DRAM"))

        def load(pool, dram, shape, dtype, name):
            t = pool.tile(shape, dtype, name=name)
            nc.sync.dma_start(out=t[:], in_=dram[:])
            return t

        # loads needed by gnn1 / the iteration-0 prep go first; the rest are
        # emitted after gnn1 so they don't hog DMA issue at startup
        oh_r = load(cpool, oh_r_d, [128, NCH * 128], BF16, "oh_r")
        oh_p = load(cpool, oh_p_d, [128, NCH * 128], BF16, "oh_p")
        ident = load(cpool, ident_d, [128, 128], F32, "ident")
        cp = load(cpool, cp_d, [ROUT, 1], F32, "cp")
        wm2 = load(cpool, wm2_d, [RIN, ROUT], F32, "wm2")
        ws2a = load(cpool, ws2a_d, [RIN + 1, ROUT], F32, "ws2a")
        rfr = [load(cpool, rfr_d[it], [128, NG * RIN], F32, f"rfr{it}")
               for it in range(ITERS)]
        if apply_mask:
            maskp = load(cpool, maskp_d, [128, NG * 128], F32, "maskp")

        # M_hat tiles (SBUF-resident, f32)
        mhat = [cpool.tile([128, 128], F32, name=f"mhat{g}") for g in range(NG)]

        # psum pools (each slot padded to one 2KB bank; 8 banks total)
        psS = stk.enter_context(tc.tile_pool(name="psS", bufs=2, space="PSUM"))
        psAgg = stk.enter_context(tc.tile_pool(name="psAgg", bufs=2, space="PSUM"))
        psU = stk.enter_context(tc.tile_pool(name="psU", bufs=2, space="PSUM"))
        psD = stk.enter_context(tc.tile_pool(name="psD", bufs=2, space="PSUM"))

        # work pools
        gath_p = stk.enter_context(tc.tile_pool(name="gath", bufs=3))
        msg_p = stk.enter_context(tc.tile_pool(name="msg", bufs=3))
        oT_p = stk.enter_context(tc.tile_pool(name="oT", bufs=9))
        small_p = stk.enter_context(tc.tile_pool(name="small", bufs=9))
        big_p = stk.enter_context(tc.tile_pool(name="big", bufs=2))
        rt_p = stk.enter_context(tc.tile_pool(name="rt", bufs=4))

        def gnn_pass(table_d, gidx_t, ewe_t, oh_t, self_aps, name,
                     f32=False, oh_cast_pool=None, ewe_from_dram=False,
                     dump_key=None):
            """One GNN layer over this core's 8 graphs.

            self_aps[g]: SBUF AP [128, 64] f32 = (x @ Ws + b) rows of graph g.
            f32: full-precision path (gnn1, feeds the huge M_hat logits);
                 gathers a [NTOT, 64] f32 table and casts one-hots to f32.
            Returns list of f32 [64, 128] transposed node outputs.
            """
            outs = []
            dt = F32 if f32 else BF16
            ew = 64 if f32 else 128
            if not ewe_from_dram:
                ewe_ap = ewe_t[:].rearrange("p (c f) -> p c f", f=64)
            NOGATHER = _os.environ.get("KERNEL_NOGATHER") == "1"
            NOMM = _os.environ.get("KERNEL_NOMM") == "1"
            for g in range(NG):
                gath = gath_p.tile([128, CPB, ew], dt,
                                   tag="gathf" if f32 else "gath")
                if NOGATHER:
                    nc.vector.memset(gath[:], 0.0)
                else:
                    nc.gpsimd.dma_gather(
                        gath[:], table_d[:],
                        gidx_t[:, g * CPB * 8:(g + 1) * CPB * 8],
                        num_idxs=EPG, num_idxs_reg=EPG, elem_size=ew,
                        single_packet=False,
                    )
                msg = msg_p.tile([128, CPB, 64], dt,
                                 tag="msgf" if f32 else "msg")
                if ewe_from_dram:
                    ewt = oh_cast_pool.tile([128, CPB, 64], F32, tag="ewt",
                                            bufs=2)
                    nc.sync.dma_start(
                        out=ewt[:],
                        in_=ewe_t[:, g * CPB * 64:(g + 1) * CPB * 64]
                        .rearrange("p (c f) -> p c f", f=64))
                    ewe_g = ewt[:]
                else:
                    ewe_g = ewe_ap[:, g * CPB:(g + 1) * CPB, :]
                nc.vector.tensor_tensor(
                    msg[:], gath[:, :, 0:64], ewe_g, op=ALU.add,
                )
                nc.vector.tensor_scalar(msg[:], msg[:], 0.0, None, op0=ALU.max)
                if DEBUG and name == "hr" and g == DBG_G:
                    nc.gpsimd.dma_start(out=dbg_d["msg"][:], in_=msg[:])
                if f32:
                    ohf = oh_cast_pool.tile([128, CPB * 128], F32, tag="ohf",
                                            bufs=2)
                    nc.vector.tensor_copy(
                        ohf[:], oh_t[:, g * CPB * 128:(g + 1) * CPB * 128])
                    oh_src = ohf
                    oh_off = 0
                else:
                    oh_src = oh_t
                    oh_off = g * CPB * 128
                agg = psAgg.tile([64, 128], F32, tag="agg")
                nc.tensor.matmul(agg[:], self_aps[g], ident[:],
                                 start=True, stop=False)
                for c in range(CPB if not NOMM else 1):
                    nc.tensor.matmul(
                        agg[:], msg[:, c, :],
                        oh_src[:, oh_off + c * 128:oh_off + (c + 1) * 128],
                        start=False, stop=(c == (CPB - 1 if not NOMM else 0)),
                    )
                oT = oT_p.tile([64, 128], F32, tag=f"oT{name}")
                nc.scalar.activation(oT[:], agg[:], AF.Relu)
                outs.append(oT)
                if DEBUG and g == DBG_G and dump_key is not None:
                    nc.gpsimd.dma_start(out=dbg_d[dump_key][:], in_=oT[:])
            return outs

        # ------------------------------------------------ startup + gnn1 ---
        with tc.tile_pool(name="tmp1", bufs=1) as tpool:
            gidx_r1 = load(tpool, gidx_r1_d, [128, NCH * 8], I16, "gidx_r1")
            gidx_p1 = load(tpool, gidx_p1_d, [128, NCH * 8], I16, "gidx_p1")
            wm1 = load(tpool, wm1_d, [FIN, DEMB], F32, "wm1")
            ws1a = load(tpool, ws1a_d, [FIN + 1, DEMB], F32, "ws1a")

            # XWm1 gather tables in DRAM, wrapped row order row'=(p*64+a).
            # xT is column-rolled per core, so the table (and gidx_*1) are in
            # rolled node order; gather results are order-insensitive.
            # The two sides share one xT slot (tag) to halve SBUF.
            selfx = {}
            for side, xT_d, tab_d in (("r", xTr_d, xwm1r_d),
                                      ("p", xTp_d, xwm1p_d)):
                xTs = tpool.tile([FIN + 1, NTOT], F32, tag="xT",
                                 name=f"xT{side}")
                nc.sync.dma_start(out=xTs[:], in_=xT_d[:])
                tab_ap = tab_d[:].rearrange("(p a) f -> p a f", p=128)
                for grp in range(8):
                    ps = psS.tile([128, 512], F32, tag="psS")
                    for j in range(8):
                        a = grp * 8 + j
                        nc.tensor.matmul(
                            ps[:, 64 * j:64 * j + 64],
                            xTs[0:FIN, 128 * a:128 * a + 128], wm1[:],
                            start=True, stop=True,
                        )
                    tg = tpool.tile([128, 8, 64], F32, tag="tabg", bufs=2)
                    nc.vector.tensor_copy(
                        tg[:], ps[:].rearrange("p (a f) -> p a f", f=64))
                    nc.sync.dma_start(
                        out=tab_ap[:, grp * 8:grp * 8 + 8, :], in_=tg[:])

                # XWs1 = x @ Ws1 + b for the core's own blocks (front columns)
                lst = []
                for g in range(NG):
                    ps = psS.tile([128, 512], F32, tag="psS")
                    nc.tensor.matmul(
                        ps[:, 0:64], xTs[:, g * 128:(g + 1) * 128],
                        ws1a[:], start=True, stop=True,
                    )
                    sx = tpool.tile([128, 64], F32, name=f"sx{side}{g}")
                    nc.scalar.copy(sx[:], ps[:, 0:64])
                    if DEBUG and side == "r" and g == DBG_G:
                        nc.gpsimd.dma_start(out=dbg_d["selfx"][:], in_=sx[:])
                    lst.append(sx)
                selfx[side] = lst

            if PHASE == "tables":
                for g in range(NG):
                    nc.vector.memset(mhat[g][:], 0.0)
            else:
                hr = gnn_pass(xwm1r_d, gidx_r1, ewe1_r_d, oh_r,
                              [t[:] for t in selfx["r"]], "hr",
                              f32=True, oh_cast_pool=tpool, ewe_from_dram=True,
                              dump_key="hrT")
                if PHASE == "gnn1r":
                    for g in range(NG):
                        ps = psS.tile([128, 512], F32, tag="psS")
                        nc.tensor.matmul(ps[:, 0:128], hr[g][:], hr[g][:],
                                         start=True, stop=True)
                        nc.vector.tensor_copy(mhat[g][:], ps[:, 0:128])
                else:
                    hp = gnn_pass(xwm1p_d, gidx_p1, ewe1_p_d, oh_p,
                                  [t[:] for t in selfx["p"]], "hp",
                                  f32=True, oh_cast_pool=tpool,
                                  ewe_from_dram=True, dump_key="hpT")
                    for g in range(NG):
                        ps = psS.tile([128, 512], F32, tag="psS")
                        nc.tensor.matmul(ps[:, 0:128], hr[g][:], hp[g][:],
                                         start=True, stop=True)
                        nc.vector.tensor_copy(mhat[g][:], ps[:, 0:128])
                if DEBUG and g == DBG_G:
                    nc.gpsimd.dma_start(out=dbg_d["mhat0"][:], in_=mhat[g][:])

        # ---------------------------------------------------- iterations ---
        def softmax(g, tag):
            negmx = small_p.tile([128, 1], F32, tag="negmx", bufs=40)
            nc.vector.tensor_reduce(negmx[:], mhat[g][:], axis=AX.X,
                                    op=ALU.max, negate=True)
            msm = big_p.tile([128, 128], F32, tag="msm", bufs=9)
            se = small_p.tile([128, 1], F32, tag="se", bufs=40)
            nc.scalar.activation(msm[:], mhat[g][:], AF.Exp,
                                 bias=negmx[:], scale=1.0, accum_out=se[:])
            rs = small_p.tile([128, 1], F32, tag="rs", bufs=40)
            nc.vector.reciprocal(rs[:], se[:])
            nc.vector.tensor_scalar(msm[:], msm[:], rs[:], None, op0=ALU.mult)
            return msm

        def emit_prep(it, g, agin_sb, agin_dram, selfp_list):
            """softmax(M_hat[g]) -> rf_p -> RWm_p (AG input shot) + RWs_p."""
            msm = softmax(g, "msm")
            if it == 0:
                nc.sync.dma_start(out=m0_d[:, g * 128:(g + 1) * 128],
                                  in_=msm[:])
            if DEBUG and g == DBG_G:
                nc.gpsimd.dma_start(out=dbg_d[f"msm{it}"][:], in_=msm[:])
            # rf_pT = rf_r^T @ M  -> [32, 128]
            psT = psS.tile([128, 512], F32, tag="psS")
            nc.tensor.matmul(
                psT[0:RIN, 0:128],
                rfr[it][:].rearrange("p (g i) -> p g i", i=RIN)[:, g, :],
                msm[:], start=True, stop=True,
            )
            rfpT = small_p.tile([RIN + 1, 128], F32, tag="rfpT")
            nc.vector.memset(rfpT[RIN:RIN + 1, :], 1.0)
            nc.scalar.copy(rfpT[0:RIN, :], psT[0:RIN, 0:128])
            if DEBUG and it == 0 and g == DBG_G:
                nc.gpsimd.dma_start(out=dbg_d["rfpT"][:], in_=rfpT[:])
            # RWm_p local -> AllGather input (bf16, 128-col padded)
            ps1 = psS.tile([128, 512], F32, tag="psS")
            nc.tensor.matmul(ps1[:, 0:64], rfpT[0:RIN, :], wm2[:],
                             start=True, stop=True)
            nc.scalar.copy(agin_sb[:, g, 0:64], ps1[:, 0:64])
            nc.sync.dma_start(out=agin_dram[:, g * 128:(g + 1) * 128],
                              in_=agin_sb[:, g, :])
            # RWs_p local (self term for gnn2-p)
            ps2 = psS.tile([128, 512], F32, tag="psS")
            nc.tensor.matmul(ps2[:, 0:64], rfpT[:], ws2a[:],
                             start=True, stop=True)
            sp = small_p.tile([128, 64], F32, tag="selfp")
            nc.scalar.copy(sp[:], ps2[:, 0:64])
            selfp_list.append(sp)

        def new_agin(it):
            agin_sb = big_p.tile([128, NG, 128], BF16, tag="agin", bufs=2)
            nc.vector.memset(agin_sb[:, :, 64:128], 0.0)
            agin_dram = dpool.tile([128, NG * 128], BF16, name=f"agin{it}")
            return agin_sb, agin_dram

        def emit_ag(it, agin_dram):
            rwmp = dpool.tile([NTOT, 128], BF16, name=f"rwmp{it}")
            nc.gpsimd.collective_compute(
                "AllGather", ALU.bypass,
                replica_groups=[list(range(NCORES))],
                ins=[agin_dram[:]], outs=[rwmp[:]],
            )
            return rwmp

        gidx_r2 = load(cpool, gidx_r2_d, [128, NCH * 8], I16, "gidx_r2")
        gidx_p2 = load(cpool, gidx_p2_d, [128, NCH * 8], I16, "gidx_p2")
        ewe2_r = load(cpool, ewe2_r_d, [128, NCH * 64], BF16, "ewe2_r")
        ewe2_p = load(cpool, ewe2_p_d, [128, NCH * 64], BF16, "ewe2_p")
        w1p = load(cpool, w1p_d, [ROUT, ROUT], F32, "w1p")
        w1n = load(cpool, w1n_d, [ROUT, ROUT], F32, "w1n")
        svec = load(cpool, svec_d, [128, 512], BF16, "svec")
        rwsr = [load(cpool, rwsr_d[it], [128, NG * 64], F32, f"rwsr{it}")
                for it in range(ITERS)]

        def emit_rside(it):
            """r-side gnn2 + folded-MLP "a_r" stacks for iteration `it`."""
            rws_ap = rwsr[it][:].rearrange("p (g f) -> p g f", f=64)
            orT = gnn_pass(rwmr_d[it], gidx_r2, ewe2_r, oh_r,
                           [rws_ap[:, g, :] for g in range(NG)], "or",
                           dump_key=f"orT{it}")
            astk = []
            for g in range(NG):
                psA = psS.tile([128, 512], F32, tag="psS")
                nc.tensor.matmul(psA[0:64, 0:128], w1p[:], orT[g][:],
                                 start=True, stop=True)
                st = small_p.tile([128, 64], F32, tag="astk", bufs=26)
                pairs = psA[0:64, 0:128].rearrange("k (u two) -> k u two",
                                                   two=2)
                nc.scalar.activation(st[0:64, :], pairs[:, :, 0],
                                     AF.Identity, bias=cp[:])
                nc.scalar.activation(st[64:128, :], pairs[:, :, 1],
                                     AF.Identity, bias=cp[:])
                if DEBUG and it == 0 and g == DBG_G:
                    nc.gpsimd.dma_start(out=dbg_d["astk"][:], in_=st[:])
                astk.append(st)
            return astk

        if PHASE in ("full", "nomlp"):
            # Launch the iteration-0 AllGather as early as possible, then use
            # its latency window for r-side gnn2 work (which is independent of
            # everything M-dependent). Each later AG window is filled by the
            # r-side work of the iteration after next.
            agin_sb, agin_dram = new_agin(0)
            selfp = []
            for g in range(NG):
                emit_prep(0, g, agin_sb, agin_dram, selfp)
            rwmp = emit_ag(0, agin_dram)

            astk_all = [emit_rside(0), emit_rside(1), None]

            for it in range(ITERS):
                astk = astk_all[it]
                opT = gnn_pass(rwmp, gidx_p2, ewe2_p, oh_p,
                               [t[:] for t in selfp], "op",
                               dump_key=f"opT{it}")
                last = it + 1 >= ITERS
                if not last:
                    agin_sb, agin_dram = new_agin(it + 1)
                    selfp = []
                for g in range(NG):
                    psd = psD.tile([128, 128], F32, tag="psD")
                    nc.tensor.matmul(psd[0:64, :], w1n[:], opT[g][:],
                                     start=True, stop=True,
                                     tile_position=(0, 0))
                    nc.tensor.matmul(psd[64:128, :], w1n[:], opT[g][:],
                                     start=True, stop=True,
                                     tile_position=(0, 64))
                    nap = big_p.tile([128, 128], BF16, tag="nap", bufs=2)
                    nc.scalar.copy(nap[:], psd[:])
                    if DEBUG and it == 0 and g == DBG_G:
                        nc.gpsimd.dma_start(out=dbg_d["nap"][:], in_=nap[:])

                    psu = psU.tile([128, 128], F32, tag="psU")
                    if PHASE != "nomlp":
                        for u in range(64):
                            strip, t = divmod(u, 16)
                            if True:
                                rt = rt_p.tile([128, 128], BF16, tag="rt")
                                if u % 8 == 7:   # offload a slice to ScalarE
                                    nc.scalar.activation(
                                        rt[:], nap[:], AF.Relu,
                                        bias=astk[g][:, u:u + 1], scale=1.0)
                                else:
                                    nc.vector.tensor_scalar(
                                        rt[:], nap[:], astk[g][:, u:u + 1],
                                        0.0, op0=ALU.add, op1=ALU.max)
                                nc.tensor.matmul(
                                    psu[32 * strip:32 * strip + 32, :],
                                    svec[:, 32 * t:32 * t + 32], rt[:],
                                    start=(t == 0), stop=(t == 15),
                                    tile_position=(0, 32 * strip),
                                )
                    if DEBUG and g == DBG_G:
                        du = big_p.tile([128, 128], F32, tag="dbgupd", bufs=1)
                        nc.vector.tensor_copy(du[:], psu[:])
                        nc.gpsimd.dma_start(out=dbg_d[f"upd{it}"][:],
                                            in_=du[:])
                    # M_hat += upd  (optionally masked / +b2)
                    if PHASE != "nomlp":
                        if b2v != 0.0 or apply_mask:
                            upd = big_p.tile([128, 128], F32, tag="upd",
                                             bufs=2)
                            if b2v != 0.0:
                                nc.vector.tensor_scalar(upd[:], psu[:], b2v,
                                                        None, op0=ALU.add)
                            else:
                                nc.vector.tensor_copy(upd[:], psu[:])
                            if apply_mask:
                                nc.vector.tensor_tensor(
                                    upd[:], upd[:],
                                    maskp[:, g * 128:(g + 1) * 128],
                                    op=ALU.mult)
                            nc.vector.tensor_tensor(mhat[g][:], mhat[g][:],
                                                    upd[:], op=ALU.add)
                        else:
                            nc.vector.tensor_tensor(mhat[g][:], mhat[g][:],
                                                    psu[:], op=ALU.add)
                    if DEBUG and g == DBG_G:
                        nc.gpsimd.dma_start(out=dbg_d[f"mh{it}"][:],
                                            in_=mhat[g][:])
                    # pipeline the next iteration's softmax/rf_p/AG-input (or
                    # the final output softmax) behind this graph's update
                    if not last:
                        emit_prep(it + 1, g, agin_sb, agin_dram, selfp)
                    else:
                        msm = softmax(g, "msmf")
                        nc.sync.dma_start(
                            out=mt_d[:, g * 128:(g + 1) * 128], in_=msm[:])
                if not last:
                    rwmp = emit_ag(it + 1, agin_dram)
                if it + 2 < ITERS:
                    astk_all[it + 2] = emit_rside(it + 2)
        else:
            for g in range(NG):
                msm = softmax(g, "msmf")
                nc.sync.dma_start(out=mt_d[:, g * 128:(g + 1) * 128],
                                  in_=msm[:])

    nc.compile()
    return nc


# ------------------------------------------------------------------ host ---

def _wrap_idx(idx_flat: np.ndarray) -> np.ndarray:
    """[n] -> [128, n//16] wrapped (i -> [i%16, i//16]), 8x replicated."""
    w = idx_flat.reshape(-1, 16).T.astype(np.int16)
    return np.tile(w, (8, 1))


def _count_cpb(edge_index) -> int:
    dst = np.asarray(edge_index[1], dtype=np.int64) % NTOT
    counts = np.bincount(dst // 128, minlength=B)
    return max(1, int(np.ceil(counts.max() / 128)))


def _prep_side(edge_index, edge_feat, We1, We2, CPB):
    """Partition/sort/pad edges by dst owner. Returns list of per-core dicts."""
    src = np.asarray(edge_index[0], dtype=np.int64) % NTOT
    dst = np.asarray(edge_index[1], dtype=np.int64) % NTOT
    ef = np.asarray(edge_feat, dtype=np.float32)
    EPG = CPB * 128
    NCH = NG * CPB
    out = []
    for c in range(NCORES):
        sel = (dst // LN) == c
        s_c, d_c, f_c = src[sel], dst[sel] - c * LN, ef[sel]
        order = np.argsort(d_c, kind="stable")
        s_c, d_c, f_c = s_c[order], d_c[order], f_c[order]
        srcs = np.zeros(NG * EPG, np.int64)
        dloc = np.zeros(NG * EPG, np.int64)
        valid = np.zeros(NG * EPG, bool)
        efp = np.zeros((NG * EPG, FE), np.float32)
        for g in range(NG):
            mk = (d_c // 128) == g
            k = int(mk.sum())
            assert k <= EPG
            srcs[g * EPG:g * EPG + k] = s_c[mk]
            dloc[g * EPG:g * EPG + k] = d_c[mk] % 128
            valid[g * EPG:g * EPG + k] = True
            efp[g * EPG:g * EPG + k] = f_c[mk]
        oh = np.zeros((NCH, 128, 128), np.float32)
        ch = np.arange(NG * EPG) // 128
        pos = np.arange(NG * EPG) % 128
        oh[ch[valid], pos[valid], dloc[valid]] = 1.0
        oh_flat = oh.transpose(1, 0, 2).reshape(128, NCH * 128)
        ewe1 = (efp @ We1).reshape(NCH, 128, 64).transpose(1, 0, 2)
        ewe2 = (efp @ We2).reshape(NCH, 128, 64).transpose(1, 0, 2)
        _ = None
        # index remaps into the three table layouts
        rolled = (srcs - c * LN) % NTOT
        roll_wrap = (rolled % 128) * (NTOT // 128) + rolled // 128
        full_wrap = (srcs % 128) * (NTOT // 128) + srcs // 128
        l = srcs % LN
        ag_wrap = (srcs // LN) * LN + (l % 128) * NG + l // 128

        def wrap_all(v):
            return np.hstack([_wrap_idx(v[g * EPG:(g + 1) * EPG])
                              for g in range(NG)])

        out.append(dict(
            gidx_roll=wrap_all(roll_wrap),
            gidx_full=wrap_all(full_wrap),
            gidx_ag=wrap_all(ag_wrap),
            oh=oh_flat.astype(BF16NP),
            ewe1=np.ascontiguousarray(ewe1.reshape(128, NCH * 64),
                                      dtype=np.float32),
            ewe2=ewe2.reshape(128, NCH * 64).astype(BF16NP),
        ))
    return out


_RF_CACHE = None

_RF_CODE = """
import os
os.environ["JAX_PLATFORMS"] = "cpu"
import numpy as np
import jax
rkey = jax.random.key(42)
out = {}
for it in range(%d):
    out[f"rf{it}"] = np.asarray(jax.random.normal(
        jax.random.fold_in(rkey, it), (%d, %d, %d), jax.numpy.float32))
np.savez(%r, **out)
"""


def _gen_rf():
    """Random features, bit-identical to the reference's jax-on-CPU threefry.

    Must run in a subprocess with JAX_PLATFORMS=cpu: with the axon/neuron
    jax plugin loaded, jax.random produces different bits even under
    jax.default_device(cpu).
    """
    global _RF_CACHE
    if _RF_CACHE is not None:
        return _RF_CACHE
    import os
    import subprocess
    import sys
    import tempfile
    import time
    path = os.path.join(tempfile.mkdtemp(), "rf.npz")
    code = _RF_CODE % (ITERS, B, N, RIN, path)
    env = dict(os.environ)
    env["JAX_PLATFORMS"] = "cpu"
    last = None
    for attempt in range(4):
        r = subprocess.run([sys.executable, "-c", code], env=env,
                           capture_output=True, text=True)
        if r.returncode == 0 and os.path.exists(path):
            break
        last = r.stderr[-1000:]
        time.sleep(2.0 * (attempt + 1))
    else:
        raise RuntimeError(f"rf generation subprocess failed: {last}")
    d = np.load(path)
    _RF_CACHE = [d[f"rf{it}"] for it in range(ITERS)]
    return _RF_CACHE


def _full_wrap_rows(T: np.ndarray) -> np.ndarray:
    """[NTOT, f] -> wrapped row order row' = (r%128)*(NTOT//128) + r//128."""
    return T.reshape(NTOT // 128, 128, -1).transpose(1, 0, 2).reshape(NTOT, -1)


def _pad128(T: np.ndarray) -> np.ndarray:
    out = np.zeros((T.shape[0], 128), np.float32)
    out[:, :T.shape[1]] = T
    return out


def kernel(x_r, edge_index_r, edge_feat_r, x_p, edge_index_p, edge_feat_p,
           r_mask, p_mask,
           g1_Ws, g1_Wm, g1_We, g1_b,
           g2_Ws, g2_Wm, g2_We, g2_b,
           mlp_W1, mlp_b1, mlp_W2, mlp_b2):
    x_r = np.asarray(x_r, np.float32)
    x_p = np.asarray(x_p, np.float32)
    g1_Ws, g1_Wm, g1_We, g1_b = [np.asarray(a, np.float32) for a in
                                 (g1_Ws, g1_Wm, g1_We, g1_b)]
    g2_Ws, g2_Wm, g2_We, g2_b = [np.asarray(a, np.float32) for a in
                                 (g2_Ws, g2_Wm, g2_We, g2_b)]
    mlp_W1, mlp_b1, mlp_W2, mlp_b2 = [np.asarray(a, np.float32) for a in
                                      (mlp_W1, mlp_b1, mlp_W2, mlp_b2)]
    r_mask = np.asarray(r_mask, bool)
    p_mask = np.asarray(p_mask, bool)

    rfs = _gen_rf()

    CPB = max(_count_cpb(edge_index_r), _count_cpb(edge_index_p))
    side_r = _prep_side(edge_index_r, edge_feat_r, g1_We, g2_We, CPB)
    side_p = _prep_side(edge_index_p, edge_feat_p, g1_We, g2_We, CPB)

    apply_mask = not (r_mask.all() and p_mask.all())
    b2v = float(mlp_b2[0])

    key = (CPB, apply_mask, b2v != 0.0)
    if key not in _prog_cache:
        _prog_cache[key] = build_program(CPB, apply_mask, b2v)
    nc = _prog_cache[key]

    # --- folded MLP constants ---
    aw = np.abs(mlp_W2[:, 0])
    s = np.sign(mlp_W2[:, 0]).astype(np.float32)
    s[s == 0] = 1.0
    w1p = mlp_W1 * aw[None, :]
    cp = (mlp_b1 * aw)[:, None].astype(np.float32)
    svec = np.zeros((128, 512), np.float32)
    for t in range(16):
        svec[0:64, 32 * t + 2 * t] = s
        svec[64:128, 32 * t + 2 * t + 1] = s

    rwmr, rws_flat, rfr_flat = [], [], []
    for it in range(ITERS):
        rf_flat = rfs[it].reshape(NTOT, RIN)
        rwmr.append(_pad128(_full_wrap_rows(rf_flat @ g2_Wm)).astype(BF16NP))
        rws_flat.append(rf_flat @ g2_Ws + g2_b[None, :])
        rfr_flat.append(rf_flat)

    rep = {
        "wm1": g1_Wm, "ws1a": np.vstack([g1_Ws, g1_b[None, :]]),
        "wm2": g2_Wm, "ws2a": np.vstack([g2_Ws, g2_b[None, :]]),
        "w1p": w1p.astype(np.float32), "w1n": (-w1p).astype(np.float32),
        "cp": cp, "svec": svec.astype(BF16NP),
        "ident": np.eye(128, dtype=np.float32),
    }
    for it in range(ITERS):
        rep[f"rwmr{it}"] = rwmr[it]

    in_maps = []
    for c in range(NCORES):
        m = dict(rep)
        for side, x in (("r", x_r), ("p", x_p)):
            xro = np.roll(x, -c * LN, axis=0)
            m[f"xT{side}"] = np.ascontiguousarray(
                np.vstack([xro.T, np.ones((1, NTOT), np.float32)]))
        d_r, d_p = side_r[c], side_p[c]
        m["gidx_r1"] = d_r["gidx_roll"]
        m["gidx_r2"] = d_r["gidx_full"]
        m["gidx_p1"] = d_p["gidx_roll"]
        m["gidx_p2"] = d_p["gidx_ag"]
        m["oh_r"] = d_r["oh"]
        m["oh_p"] = d_p["oh"]
        m["ewe1_r"] = d_r["ewe1"]
        m["ewe1_p"] = d_p["ewe1"]
        m["ewe2_r"] = d_r["ewe2"]
        m["ewe2_p"] = d_p["ewe2"]
        loc = slice(c * LN, (c + 1) * LN)
        for it in range(ITERS):
            m[f"rfr{it}"] = np.ascontiguousarray(
                rfr_flat[it][loc].reshape(NG, 128, RIN)
                .transpose(1, 0, 2).reshape(128, NG * RIN)).astype(np.float32)
            m[f"rwsr{it}"] = np.ascontiguousarray(
                rws_flat[it][loc].reshape(NG, 128, 64)
                .transpose(1, 0, 2).reshape(128, NG * 64)).astype(np.float32)
        if apply_mask:
            rm = r_mask[c * NG:(c + 1) * NG].astype(np.float32)  # [NG, 128]
            pm = p_mask[c * NG:(c + 1) * NG].astype(np.float32)
            mp = rm[:, :, None] * pm[:, None, :]                 # [NG, r, p]
            m["maskp"] = np.ascontiguousarray(
                mp.transpose(1, 0, 2).reshape(128, NG * 128))
        in_maps.append(m)

    global LAST_RESULTS, LAST_IN_MAPS
    LAST_IN_MAPS = in_maps
    res = _run(nc, in_maps, list(range(NCORES)))
    LAST_RESULTS = res
    m0 = np.concatenate(
        [np.asarray(res.results[c]["m0"]).reshape(128, NG, 128)
         .transpose(1, 0, 2) for c in range(NCORES)], axis=0).reshape(NTOT, N)
    mt = np.concatenate(
        [np.asarray(res.results[c]["mt"]).reshape(128, NG, 128)
         .transpose(1, 0, 2) for c in range(NCORES)], axis=0).reshape(NTOT, N)
    return (np.ascontiguousarray(m0, dtype=np.float32),
            np.ascontiguousarray(mt, dtype=np.float32))
